# revision 2
# baseline (speedup 1.0000x reference)
"""MetaLayer kernel: data-parallel over B=16 samples across 8 NeuronCores.

Strategy: shard the vmapped batch dim (B=16 -> 2 samples/core), replicate all
parameters. Each core runs the per-sample inner-loss forward+backward, the
fast-weight update, the return_x swiglu and the final layernorm, via jax
compiled per-device (axon PJRT -> NEFF on trn2).
"""

import numpy as np

D = 512
B = 16
T = 1024
NCORES = 8


def _build():
    import jax
    import jax.numpy as jnp

    def _silu(x):
        return x * jax.nn.sigmoid(x)

    def _swiglu(x, W1, b1, W2, b2):
        h = x @ W1 + b1
        a, b = jnp.split(h, 2, axis=-1)
        return (_silu(a) * b) @ W2 + b2

    def _forward_shard(x, W1, b1, W2, b2, L, Wp1, bp1, Wp2, bp2,
                       uW1, ub1, uW2, ub2, lr, gamma, beta):
        # x: (2*Bs, T, d) shard; same math as reference but on the shard.
        twoB, T_, d = x.shape
        xb = x.reshape(2, twoB // 2, T_, d).transpose(1, 0, 2, 3)

        def loss(in_ff, xs):
            w1, bb1, w2, bb2 = in_ff
            h = _swiglu(xs, w1, bb1, w2, bb2)
            y = jnp.einsum('atd,ade->te', h, L)
            p = _swiglu(y, Wp1, bp1, Wp2, bp2)
            a2, b2_ = jnp.split(p, 2, axis=-1)
            dot = jnp.sum(a2 * b2_, axis=-1)
            na = jnp.linalg.norm(a2, axis=-1)
            nb = jnp.linalg.norm(b2_, axis=-1)
            cos = dot / (jnp.maximum(na, 1e-8) * jnp.maximum(nb, 1e-8))
            return jnp.mean(2.0 + 2.0 * cos)

        la = jnp.abs(lr)

        def per_sample(xs):
            g = jax.grad(loss)((W1, b1, W2, b2), xs)
            return _swiglu(xs, uW1 - la * g[0], ub1 - la * g[1],
                           uW2 - la * g[2], ub2 - la * g[3])

        out = jax.vmap(per_sample)(xb)
        out = out.transpose(1, 0, 2, 3).reshape(twoB, T_, d)
        mu = jnp.mean(out, axis=-1, keepdims=True)
        var = jnp.var(out, axis=-1, keepdims=True)
        return (out - mu) / jnp.sqrt(var + 1e-5) * gamma + beta

    return jax, jnp, _forward_shard


def kernel(x, W1, b1, W2, b2, L, Wp1, bp1, Wp2, bp2,
           uW1, ub1, uW2, ub2, lr, gamma, beta):
    jax, jnp, fwd = _build()

    devs = jax.devices()[:NCORES]
    params = (W1, b1, W2, b2, L, Wp1, bp1, Wp2, bp2,
              uW1, ub1, uW2, ub2, lr, gamma, beta)

    # Shard samples: sample i is rows (i, 16+i) of x. Core c gets samples
    # 2c, 2c+1 -> x rows [2c, 2c+1] and [16+2c, 16+2c+1].
    Bs = B // NCORES  # samples per core
    fn = jax.jit(fwd)

    outs = []
    shards = []
    for c in range(NCORES):
        rows = list(range(Bs * c, Bs * (c + 1))) + \
               list(range(B + Bs * c, B + Bs * (c + 1)))
        shards.append(np.ascontiguousarray(x[rows]))

    # One jit, dispatched async to each device; NEFF is compiled once and
    # reused (identical HLO on every core).
    futures = []
    for c, dev in enumerate(devs):
        args = [jax.device_put(shards[c], dev)] + \
               [jax.device_put(p, dev) for p in params]
        futures.append(fn(*args))
    for c in range(NCORES):
        outs.append(np.asarray(futures[c]))

    out = np.empty((2 * B, T, D), np.float32)
    for c in range(NCORES):
        rows = list(range(Bs * c, Bs * (c + 1))) + \
               list(range(B + Bs * c, B + Bs * (c + 1)))
        out[rows] = outs[c]
    return out


if __name__ == "__main__":
    import reference
    inputs = reference.setup_inputs()
    inputs = {k: np.asarray(v) for k, v in inputs.items()}
    exp = np.asarray(reference.reference(**inputs))
    act = kernel(**inputs)
    err = np.abs(act - exp).max() / (np.abs(exp).max() + 1e-9)
    print("Relative error:", err)
